# revision 6
# baseline (speedup 1.0000x reference)
"""Trainium2 Bass kernel for a GNN message-passing layer.

Reference computation (per graph):
    src,dst = edge_indices
    h   = gelu(concat(x[src], x[dst], e) @ W1m + b1m)          # [E, H]
    msg = h @ W2m + b2m                                        # [E, H]
    agg = segment_sum(msg, dst)                                # [N, H]
    u   = gelu(concat(x, agg) @ W1u + b1u)                     # [N, H]
    out = u @ W2u + b2u                                        # [N, D]

Device strategy (8 cores = 2 graphs x 4 dst-ranges):
  - By linearity, W2m is applied AFTER aggregation: agg = segsum(h) @ W2m + deg*b2m.
  - W1m splits into per-node projections Psrc = x@W1m[:D], Pdst = x@W1m[D:2D]
    (computed once per node, stored bf16 in DRAM, row-gathered per edge) plus a
    dense per-edge projection eproj = e@W1m[2D:] + b1m.
  - h = gelu(Psrc[src] + Pdst[dst] + eproj), token-major [128e, H] tiles.
  - Scatter-add via one-hot matmul: sel[e,n] = (dst_rel[e]==n); per node block
    aggT[h,n] += h_chunk.T @ sel accumulated in PSUM -> feature-major aggregates.
  - Host pre-sorts each graph's edges by destination block (128 nodes/block,
    20 blocks/core) and pads each block's edge count to a multiple of 128.
"""

import os
import sys

sys.path.insert(0, "/opt/trn_rl_repo")

import numpy as np
import ml_dtypes

import concourse.bacc as bacc
import concourse.mybir as mybir
import concourse.tile as tile
from concourse.bass_utils import run_bass_kernel_spmd

BF16 = ml_dtypes.bfloat16

B, N, E = 2, 10000, 160000
D, F, H = 128, 64, 256
NCORES = 8
CPG = NCORES // B          # cores per graph = 4
NBLK = 20                  # node blocks per core
BLK = 128                  # nodes per block
NSLICE = NBLK * BLK        # 2560 nodes per core
NPAD = CPG * NSLICE        # 10240 padded nodes per graph

f32 = mybir.dt.float32
bf16 = mybir.dt.bfloat16
i16 = mybir.dt.int16

_BUILD_CACHE = {}


def _build(k_blk):
    """Build the SPMD single-core program. k_blk = 128-edge chunks per node block."""
    nchunk = NBLK * k_blk
    ecap = nchunk * 128

    nc = bacc.Bacc(None)

    # ---- external inputs (per-core) ----
    nft = nc.dram_tensor("nft", [D, NPAD], bf16, kind="ExternalInput")
    nfs = nc.dram_tensor("nfs", [D, NSLICE], bf16, kind="ExternalInput")
    eft = nc.dram_tensor("eft", [F + 1, ecap], bf16, kind="ExternalInput")
    gidx = nc.dram_tensor("gidx", [128, ecap * 2 // 16], i16, kind="ExternalInput")
    dstrel = nc.dram_tensor("dstrel", [128, nchunk], bf16, kind="ExternalInput")
    degrow = nc.dram_tensor("degrow", [1, NSLICE], bf16, kind="ExternalInput")
    w1e = nc.dram_tensor("w1e", [F + 1, H], bf16, kind="ExternalInput")
    wsrc = nc.dram_tensor("wsrc", [D, H], bf16, kind="ExternalInput")
    wdst = nc.dram_tensor("wdst", [D, H], bf16, kind="ExternalInput")
    # weight k-chunks packed side-by-side: [128, nchunks*cols]
    w2m = nc.dram_tensor("w2m", [128, 2 * H], bf16, kind="ExternalInput")
    b2mr = nc.dram_tensor("b2mr", [1, H], bf16, kind="ExternalInput")
    w1u = nc.dram_tensor("w1u", [128, 3 * H], bf16, kind="ExternalInput")
    b1uc = nc.dram_tensor("b1uc", [128, 2], f32, kind="ExternalInput")
    w2u = nc.dram_tensor("w2u", [128, 2 * D], bf16, kind="ExternalInput")
    b2ur = nc.dram_tensor("b2ur", [1, D], bf16, kind="ExternalInput")
    onesr = nc.dram_tensor("onesr", [1, 128], bf16, kind="ExternalInput")
    iotat = nc.dram_tensor("iotat", [128, 128], bf16, kind="ExternalInput")

    out = nc.dram_tensor("out", [NSLICE, D], f32, kind="ExternalOutput")

    with tile.TileContext(nc) as tc:
        with (
            tc.tile_pool(name="const", bufs=1) as cpool,
            tc.tile_pool(name="dram", bufs=1, space="DRAM") as dpool,
            tc.tile_pool(name="eftp", bufs=3) as eftp,
            tc.tile_pool(name="gath", bufs=2) as gpool,
            tc.tile_pool(name="sel", bufs=4) as selp,
            tc.tile_pool(name="hwork", bufs=4) as hp,
            tc.tile_pool(name="cp", bufs=4) as cp,
            tc.tile_pool(name="psA", bufs=3, space="PSUM") as psA,
            tc.tile_pool(name="agg", bufs=4, space="PSUM") as psG,
        ):
            # ---- load constants / persistent tensors into SBUF ----
            def load(dram_t, shape, dtype):
                t = cpool.tile(shape, dtype, tag=dram_t.name)
                nc.sync.dma_start(out=t[:], in_=dram_t[:])
                return t

            nft_s = load(nft, [D, NPAD], bf16)
            nfs_s = load(nfs, [D, NSLICE], bf16)
            gidx_s = load(gidx, [128, ecap * 2 // 16], i16)
            dstrel_s = load(dstrel, [128, nchunk], bf16)
            degrow_s = load(degrow, [1, NSLICE], bf16)
            w1e_s = load(w1e, [F + 1, H], bf16)
            wsrc_s = load(wsrc, [D, H], bf16)
            wdst_s = load(wdst, [D, H], bf16)
            w2m_s = load(w2m, [128, 2 * H], bf16)
            b2mr_s = load(b2mr, [1, H], bf16)
            w1u_s = load(w1u, [128, 3 * H], bf16)
            b1uc_s = load(b1uc, [128, 2], f32)
            w2u_s = load(w2u, [128, 2 * D], bf16)
            b2ur_s = load(b2ur, [1, D], bf16)
            onesr_s = load(onesr, [1, 128], bf16)
            iotat_s = load(iotat, [128, 128], bf16)

            # projection tables in DRAM: rows [0,NPAD) = Psrc, [NPAD,NPAD+NSLICE) = Pdst
            pall = dpool.tile([NPAD + NSLICE, H], bf16)

            # ---- stage 1: projection tables ----
            for nb in range(NPAD // 128):
                ps = psA.tile([128, H], f32, tag="psA")
                nc.tensor.matmul(
                    out=ps[:], lhsT=nft_s[:, nb * 128:(nb + 1) * 128],
                    rhs=wsrc_s[:], start=True, stop=True,
                )
                pc = cp.tile([128, H], bf16, tag="cp")
                nc.vector.tensor_copy(out=pc[:], in_=ps[:])
                nc.sync.dma_start(out=pall[nb * 128:(nb + 1) * 128, :], in_=pc[:])
            for nb in range(NBLK):
                ps = psA.tile([128, H], f32, tag="psA")
                nc.tensor.matmul(
                    out=ps[:], lhsT=nfs_s[:, nb * 128:(nb + 1) * 128],
                    rhs=wdst_s[:], start=True, stop=True,
                )
                pc = cp.tile([128, H], bf16, tag="cp")
                nc.vector.tensor_copy(out=pc[:], in_=ps[:])
                nc.sync.dma_start(
                    out=pall[NPAD + nb * 128:NPAD + (nb + 1) * 128, :], in_=pc[:]
                )

            # persistent per-core accumulators (feature-major, bf16)
            aggT0 = cpool.tile([128, NSLICE], bf16, tag="aggT0")
            aggT1 = cpool.tile([128, NSLICE], bf16, tag="aggT1")
            agfT0 = cpool.tile([128, NSLICE], bf16, tag="agfT0")
            agfT1 = cpool.tile([128, NSLICE], bf16, tag="agfT1")
            u0 = cpool.tile([128, NSLICE], bf16, tag="u0")
            u1 = cpool.tile([128, NSLICE], bf16, tag="u1")

            # ---- stage 2: edge pipeline ----
            GCH = 4  # chunks per dma_gather (1024 indices)
            for blk in range(NBLK):
                # gather Psrc[src]/Pdst[dst] for this block's edges
                gt = gpool.tile([128, 2 * k_blk, H], bf16, tag="gath")
                for g0 in range(0, k_blk, GCH):
                    g1 = min(g0 + GCH, k_blk)
                    nidx = (g1 - g0) * 256
                    c0 = blk * k_blk + g0
                    nc.gpsimd.dma_gather(
                        gt[:, 2 * g0:2 * g1, :],
                        pall[:],
                        gidx_s[:, c0 * 16:c0 * 16 + nidx // 16],
                        num_idxs=nidx,
                        num_idxs_reg=nidx,
                        elem_size=H,
                    )
                # this block's edge features (feature-major, with ones row)
                et = eftp.tile([F + 1, k_blk * 128], bf16, tag="eft")
                nc.sync.dma_start(
                    out=et[:], in_=eft[:, blk * k_blk * 128:(blk + 1) * k_blk * 128]
                )

                ag0 = psG.tile([128, 128], f32, tag="agg")
                ag1 = psG.tile([128, 128], f32, tag="agg")
                for k in range(k_blk):
                    ci = blk * k_blk + k
                    # selection matrix sel[e, n] = (dst_rel[e] == n)
                    sel = selp.tile([128, 128], bf16, tag="sel")
                    nc.vector.tensor_tensor(
                        out=sel[:],
                        in0=dstrel_s[:, ci:ci + 1].to_broadcast([128, 128]),
                        in1=iotat_s[:],
                        op=mybir.AluOpType.is_equal,
                    )
                    # eproj = eT.T @ W1e (+b1m via ones row)
                    pe = psA.tile([128, H], f32, tag="psA")
                    nc.tensor.matmul(
                        out=pe[:], lhsT=et[:, k * 128:(k + 1) * 128],
                        rhs=w1e_s[:], start=True, stop=True,
                    )
                    # s = Psrc[src] + Pdst[dst] + eproj ; h = gelu(s)
                    t1 = hp.tile([128, H], bf16, tag="t1")
                    nc.vector.tensor_add(
                        out=t1[:], in0=gt[:, 2 * k, :], in1=gt[:, 2 * k + 1, :]
                    )
                    s = hp.tile([128, H], bf16, tag="s")
                    nc.vector.tensor_add(out=s[:], in0=t1[:], in1=pe[:])
                    h = hp.tile([128, H], bf16, tag="h")
                    nc.scalar.activation(
                        out=h[:], in_=s[:], func=mybir.ActivationFunctionType.Gelu_apprx_tanh
                    )
                    # scatter: aggT[:, blk] += h.T @ sel
                    nc.tensor.matmul(
                        out=ag0[:], lhsT=h[:, 0:128], rhs=sel[:],
                        start=(k == 0), stop=(k == k_blk - 1),
                    )
                    nc.tensor.matmul(
                        out=ag1[:], lhsT=h[:, 128:256], rhs=sel[:],
                        start=(k == 0), stop=(k == k_blk - 1),
                    )
                csl = slice(blk * 128, (blk + 1) * 128)
                nc.vector.tensor_copy(out=aggT0[:, csl], in_=ag0[:])
                nc.vector.tensor_copy(out=aggT1[:, csl], in_=ag1[:])

            # ---- stage 3: per-node MLPs (feature-major, 512-node groups) ----
            for g5 in range(NSLICE // 512):
                sl = slice(g5 * 512, (g5 + 1) * 512)
                for o in range(2):
                    osl = slice(o * 128, (o + 1) * 128)
                    # aggfinal = aggT.T@W2m + deg*b2m   (feature-major out)
                    pa = psA.tile([128, 512], f32, tag="psA")
                    nc.tensor.matmul(out=pa[:], lhsT=w2m_s[:, 0 * H + o * 128:0 * H + (o + 1) * 128],
                                     rhs=aggT0[:, sl], start=True, stop=False)
                    nc.tensor.matmul(out=pa[:], lhsT=w2m_s[:, 1 * H + o * 128:1 * H + (o + 1) * 128],
                                     rhs=aggT1[:, sl], start=False, stop=False)
                    nc.tensor.matmul(out=pa[:], lhsT=b2mr_s[:, osl],
                                     rhs=degrow_s[:, sl], start=False, stop=True)
                    dst_t = agfT0 if o == 0 else agfT1
                    nc.vector.tensor_copy(out=dst_t[:, sl], in_=pa[:])
                for o in range(2):
                    osl = slice(o * 128, (o + 1) * 128)
                    # u = gelu(concat(x, aggfinal) @ W1u + b1u)
                    pu = psA.tile([128, 512], f32, tag="psA")
                    nc.tensor.matmul(out=pu[:], lhsT=w1u_s[:, 0 * H + o * 128:0 * H + (o + 1) * 128],
                                     rhs=nfs_s[:, sl], start=True, stop=False)
                    nc.tensor.matmul(out=pu[:], lhsT=w1u_s[:, 1 * H + o * 128:1 * H + (o + 1) * 128],
                                     rhs=agfT0[:, sl], start=False, stop=False)
                    nc.tensor.matmul(out=pu[:], lhsT=w1u_s[:, 2 * H + o * 128:2 * H + (o + 1) * 128],
                                     rhs=agfT1[:, sl], start=False, stop=True)
                    dst_t = u0 if o == 0 else u1
                    nc.scalar.activation(
                        out=dst_t[:, sl], in_=pu[:],
                        func=mybir.ActivationFunctionType.Gelu_apprx_tanh,
                        bias=b1uc_s[:, o:o + 1],
                    )
            # out = u @ W2u + b2u   (token-major out per node block)
            for blk in range(NBLK):
                csl = slice(blk * 128, (blk + 1) * 128)
                po = psA.tile([128, 128], f32, tag="psA")
                nc.tensor.matmul(out=po[:], lhsT=u0[:, csl], rhs=w2u_s[:, 0:D],
                                 start=True, stop=False)
                nc.tensor.matmul(out=po[:], lhsT=u1[:, csl], rhs=w2u_s[:, D:2 * D],
                                 start=False, stop=False)
                nc.tensor.matmul(out=po[:], lhsT=onesr_s[:], rhs=b2ur_s[:],
                                 start=False, stop=True)
                oc = cp.tile([128, 128], f32, tag="ocp")
                nc.vector.tensor_copy(out=oc[:], in_=po[:])
                nc.sync.dma_start(out=out[csl, :], in_=oc[:])

    nc.finalize()
    return nc


def _prep_core_inputs(g, r, node_features, edge_indices, edge_features, k_blk, shared):
    """Host-side shard prep for core (graph g, dst-range r)."""
    nchunk = NBLK * k_blk
    ecap = nchunk * 128
    dst = edge_indices[g, :, 1]
    src = edge_indices[g, :, 0]
    lo, hi = r * NSLICE, (r + 1) * NSLICE

    mask = (dst >= lo) & (dst < hi)
    eid = np.nonzero(mask)[0]
    dloc = dst[eid] - lo
    blk_of = dloc // BLK
    order = np.argsort(blk_of, kind="stable")
    eid = eid[order]
    dloc = dloc[order]
    blk_of = blk_of[order]
    counts = np.bincount(blk_of, minlength=NBLK)

    # slot layout: block b occupies [b*k_blk*128, b*k_blk*128 + counts[b])
    slot = np.zeros(ecap, dtype=np.int64) - 1
    srcpad = np.zeros(ecap, dtype=np.int64)            # gather idx 0 for pads
    dstpad = np.zeros(ecap, dtype=np.int64)            # gather Pdst row 0 for pads
    drel = np.full(ecap, -1.0, dtype=np.float64)       # -1 => sel column all-zero
    epos = 0
    for b in range(NBLK):
        cnt = counts[b]
        s0 = b * k_blk * 128
        ids = eid[epos:epos + cnt]
        srcpad[s0:s0 + cnt] = src[ids]
        dstpad[s0:s0 + cnt] = dloc[epos:epos + cnt]
        drel[s0:s0 + cnt] = (dloc[epos:epos + cnt] - b * BLK).astype(np.float64)
        slot[s0:s0 + cnt] = ids
        epos += cnt

    # edge features (feature-major + ones row), zeros for pads
    eftc = np.zeros((F + 1, ecap), dtype=BF16)
    valid = slot >= 0
    eftc[:F, valid] = edge_features[g, slot[valid], :].T.astype(BF16)
    eftc[F, :] = BF16(1.0)

    # gather indices: per chunk, 128 src rows then 128 dst rows (into Pall)
    gflat = np.empty(ecap * 2, dtype=np.int64)
    sc = srcpad.reshape(nchunk, 128)
    dc = dstpad.reshape(nchunk, 128) + NPAD
    inter = np.stack([sc, dc], axis=1).reshape(-1)     # [nchunk*2*128]
    gflat[:] = inter
    # idx block wrapped in 16 partitions, replicated for each of the 8 Q7 cores
    gidxc = np.tile(gflat.astype(np.int16).reshape(-1, 16).T, (8, 1))

    # dst_rel per chunk column
    drelc = np.ascontiguousarray(drel.reshape(nchunk, 128).T).astype(BF16)

    deg = np.bincount(dloc, minlength=NSLICE).astype(np.float64)
    degc = deg[None, :].astype(BF16)

    inp = dict(shared)
    inp["nft"] = shared["_nftg"][g]
    inp["nfs"] = np.ascontiguousarray(shared["_nftg"][g][:, lo:hi])
    inp["eft"] = eftc
    inp["gidx"] = gidxc
    inp["dstrel"] = drelc
    inp["degrow"] = degc
    return {k: v for k, v in inp.items() if not k.startswith("_")}


def kernel(node_features, edge_indices, edge_features,
           W1m, b1m, W2m, b2m, W1u, b1u, W2u, b2u):
    node_features = np.asarray(node_features)
    edge_indices = np.asarray(edge_indices)
    edge_features = np.asarray(edge_features)

    # chunks per node block: driven by the actual max block occupancy
    dst = edge_indices[..., 1]
    blk_id = (np.arange(B)[:, None] * (NPAD // BLK)) + dst // BLK
    counts = np.bincount(blk_id.reshape(-1), minlength=B * NPAD // BLK)
    k_blk = int(np.ceil(counts.max() / 128.0))

    if k_blk not in _BUILD_CACHE:
        _BUILD_CACHE[k_blk] = _build(k_blk)
    nc = _BUILD_CACHE[k_blk]

    # node features transposed + padded, bf16, per graph
    nftg = np.zeros((B, D, NPAD), dtype=BF16)
    for g in range(B):
        nftg[g, :, :N] = np.asarray(node_features[g]).T.astype(BF16)

    iota = np.broadcast_to(np.arange(128, dtype=np.float32), (128, 128))
    shared = {
        "_nftg": nftg,
        "w1e": np.concatenate([np.asarray(W1m)[2 * D:], np.asarray(b1m)[None, :]],
                              axis=0).astype(BF16),
        "wsrc": np.asarray(W1m)[:D].astype(BF16),
        "wdst": np.asarray(W1m)[D:2 * D].astype(BF16),
        "w2m": np.asarray(W2m).reshape(2, 128, H).transpose(1, 0, 2).reshape(128, 2 * H).astype(BF16),
        "b2mr": np.asarray(b2m)[None, :].astype(BF16),
        "w1u": np.asarray(W1u).reshape(3, 128, H).transpose(1, 0, 2).reshape(128, 3 * H).astype(BF16),
        "b1uc": np.asarray(b1u).reshape(2, 128).T.astype(np.float32).copy(),
        "w2u": np.asarray(W2u).reshape(2, 128, D).transpose(1, 0, 2).reshape(128, 2 * D).astype(BF16),
        "b2ur": np.asarray(b2u)[None, :].astype(BF16),
        "onesr": np.ones((1, 128), dtype=BF16),
        "iotat": iota.astype(BF16),
    }

    in_maps = []
    for c in range(NCORES):
        g, r = c // CPG, c % CPG
        in_maps.append(_prep_core_inputs(
            g, r, node_features, edge_indices, edge_features, k_blk, shared))

    global _LAST_IN_MAPS
    _LAST_IN_MAPS = in_maps
    res = run_bass_kernel_spmd(nc, in_maps, core_ids=list(range(NCORES)))

    outp = np.zeros((B, NPAD, D), dtype=np.float32)
    for c in range(NCORES):
        g, r = c // CPG, c % CPG
        outp[g, r * NSLICE:(r + 1) * NSLICE, :] = res.results[c]["out"]
    return outp[:, :N, :]


# revision 14
# speedup vs baseline: 1.5237x; 1.5237x over previous
"""Trainium2 Bass kernel for a GNN message-passing layer.

Reference computation (per graph):
    src,dst = edge_indices
    h   = gelu(concat(x[src], x[dst], e) @ W1m + b1m)          # [E, H]
    msg = h @ W2m + b2m                                        # [E, H]
    agg = segment_sum(msg, dst)                                # [N, H]
    u   = gelu(concat(x, agg) @ W1u + b1u)                     # [N, H]
    out = u @ W2u + b2u                                        # [N, D]

Device strategy (8 cores = 2 graphs x 4 dst-ranges):
  - By linearity, W2m is applied AFTER aggregation: agg = segsum(h) @ W2m + deg*b2m.
  - W1m splits into per-node projections Psrc = x@W1m[:D], Pdst = x@W1m[D:2D]
    (computed once per node, stored bf16 in DRAM, row-gathered per edge) plus a
    dense per-edge projection eproj = e@W1m[2D:] + b1m.
  - h = gelu(Psrc[src] + Pdst[dst] + eproj), token-major [128e, H] tiles.
  - Scatter-add via one-hot matmul: sel[e,n] = (dst_rel[e]==n); per node block
    aggT[h,n] += h_chunk.T @ sel accumulated in PSUM -> feature-major aggregates.
  - Host pre-sorts each graph's edges by destination block (128 nodes/block,
    20 blocks/core) and pads each block's edge count to a multiple of 128.
"""

import os
import sys

sys.path.insert(0, "/opt/trn_rl_repo")

import numpy as np
import ml_dtypes

import concourse.bacc as bacc
import concourse.mybir as mybir
import concourse.tile as tile
from concourse.bass_utils import run_bass_kernel_spmd

BF16 = ml_dtypes.bfloat16

B, N, E = 2, 10000, 160000
D, F, H = 128, 64, 256
NCORES = 8
CPG = NCORES // B          # cores per graph = 4
NBLK = 20                  # node blocks per core
BLK = 128                  # nodes per block
NSLICE = NBLK * BLK        # 2560 nodes per core
NPAD = CPG * NSLICE        # 10240 padded nodes per graph

f32 = mybir.dt.float32
bf16 = mybir.dt.bfloat16
i16 = mybir.dt.int16

_BUILD_CACHE = {}


def _build(k_blk):
    """Build the SPMD single-core program. k_blk = 128-edge chunks per node block."""
    nchunk = NBLK * k_blk
    ecap = nchunk * 128

    nc = bacc.Bacc(None, num_swdge_queues=2)

    # ---- external inputs (per-core) ----
    nft = nc.dram_tensor("nft", [D, NPAD], bf16, kind="ExternalInput")
    nfs = nc.dram_tensor("nfs", [D, NSLICE], bf16, kind="ExternalInput")
    eft = nc.dram_tensor("eft", [F + 1, ecap], bf16, kind="ExternalInput")
    gidx = nc.dram_tensor("gidx", [128, ecap // 16], i16, kind="ExternalInput")
    dstrel = nc.dram_tensor("dstrel", [128, nchunk], bf16, kind="ExternalInput")
    dstrep = nc.dram_tensor("dstrep", [128, ecap], bf16, kind="ExternalInput")
    iotac = nc.dram_tensor("iotac", [128, 1], bf16, kind="ExternalInput")
    degrow = nc.dram_tensor("degrow", [1, NSLICE], bf16, kind="ExternalInput")
    w1e = nc.dram_tensor("w1e", [F + 1, H], bf16, kind="ExternalInput")
    wsrc = nc.dram_tensor("wsrc", [D, H], bf16, kind="ExternalInput")
    wdst = nc.dram_tensor("wdst", [D, H], bf16, kind="ExternalInput")
    # weight k-chunks packed side-by-side: [128, nchunks*cols]
    w2m = nc.dram_tensor("w2m", [128, 2 * H], bf16, kind="ExternalInput")
    b2mr = nc.dram_tensor("b2mr", [1, H], bf16, kind="ExternalInput")
    w1u = nc.dram_tensor("w1u", [128, 3 * H], bf16, kind="ExternalInput")
    b1uc = nc.dram_tensor("b1uc", [128, 2], f32, kind="ExternalInput")
    w2u = nc.dram_tensor("w2u", [128, 2 * D], bf16, kind="ExternalInput")
    b2ur = nc.dram_tensor("b2ur", [1, D], bf16, kind="ExternalInput")
    onesr = nc.dram_tensor("onesr", [1, 128], bf16, kind="ExternalInput")
    iotat = nc.dram_tensor("iotat", [128, 128], bf16, kind="ExternalInput")

    out = nc.dram_tensor("out", [NSLICE, D], f32, kind="ExternalOutput")

    with tile.TileContext(nc) as tc:
        with (
            tc.tile_pool(name="const", bufs=1) as cpool,
            tc.tile_pool(name="dram", bufs=1, space="DRAM") as dpool,
            tc.tile_pool(name="eftp", bufs=3) as eftp,
            tc.tile_pool(name="gath", bufs=2) as gpool,
            tc.tile_pool(name="sel", bufs=4) as selp,
            tc.tile_pool(name="hwork", bufs=4) as hp,
            tc.tile_pool(name="cp", bufs=4) as cp,
            tc.tile_pool(name="psA", bufs=2, space="PSUM") as psA,
            tc.tile_pool(name="agg", bufs=4, space="PSUM") as psG,
        ):
            # ---- load constants / persistent tensors into SBUF ----
            def load(dram_t, shape, dtype):
                t = cpool.tile(shape, dtype, tag=dram_t.name)
                nc.sync.dma_start(out=t[:], in_=dram_t[:])
                return t

            nft_s = load(nft, [D, NPAD], bf16)
            nfs_s = load(nfs, [D, NSLICE], bf16)
            gidx_s = load(gidx, [128, ecap // 16], i16)
            dstrel_s = load(dstrel, [128, nchunk], bf16)
            iotac_s = load(iotac, [128, 1], bf16)
            degrow_s = load(degrow, [1, NSLICE], bf16)
            w1e_s = load(w1e, [F + 1, H], bf16)
            wsrc_s = load(wsrc, [D, H], bf16)
            wdst_s = load(wdst, [D, H], bf16)
            w2m_s = load(w2m, [128, 2 * H], bf16)
            b2mr_s = load(b2mr, [1, H], bf16)
            w1u_s = load(w1u, [128, 3 * H], bf16)
            b1uc_s = load(b1uc, [128, 2], f32)
            w2u_s = load(w2u, [128, 2 * D], bf16)
            b2ur_s = load(b2ur, [1, D], bf16)
            onesr_s = load(onesr, [1, 128], bf16)
            iotat_s = load(iotat, [128, 128], bf16)

            # Psrc table in DRAM (gather source); Pdst lives in SBUF (block-local)
            pall = dpool.tile([NPAD, H], bf16)
            pdst_sb = cpool.tile([128, NBLK * H], bf16, tag="pdst")

            # ---- stage 1: projection tables ----
            for nb in range(NBLK):
                ps = psA.tile([128, H], f32, tag="psA")
                nc.tensor.matmul(
                    out=ps[:], lhsT=nfs_s[:, nb * 128:(nb + 1) * 128],
                    rhs=wdst_s[:], start=True, stop=True,
                )
                nc.scalar.copy(out=pdst_sb[:, nb * H:(nb + 1) * H], in_=ps[:])
            for nb in range(NPAD // 128):
                ps = psA.tile([128, H], f32, tag="psA")
                nc.tensor.matmul(
                    out=ps[:], lhsT=nft_s[:, nb * 128:(nb + 1) * 128],
                    rhs=wsrc_s[:], start=True, stop=True,
                )
                pc = cp.tile([128, H], bf16, tag="cp")
                if nb % 2 == 0:
                    nc.vector.tensor_copy(out=pc[:], in_=ps[:])
                else:
                    nc.scalar.copy(out=pc[:], in_=ps[:])
                nc.sync.dma_start(out=pall[nb * 128:(nb + 1) * 128, :], in_=pc[:])

            # persistent per-core accumulators (feature-major, bf16)
            aggT0 = cpool.tile([128, NSLICE], bf16, tag="aggT0")
            aggT1 = cpool.tile([128, NSLICE], bf16, tag="aggT1")
            agfT0 = cpool.tile([128, NSLICE], bf16, tag="agfT0")
            agfT1 = cpool.tile([128, NSLICE], bf16, tag="agfT1")
            u0 = cpool.tile([128, NSLICE], bf16, tag="u0")
            u1 = cpool.tile([128, NSLICE], bf16, tag="u1")

            # ---- stage 2: edge pipeline (groups of GCH 128-edge chunks) ----
            GCH = 4
            qn = 0
            for blk in range(NBLK):
                # this block's edge features (feature-major, with ones row)
                et = eftp.tile([F + 1, k_blk * 128], bf16, tag="eft")
                nc.sync.dma_start(
                    out=et[:], in_=eft[:, blk * k_blk * 128:(blk + 1) * k_blk * 128]
                )
                # replicated dst_rel rows for this block (sel_T operand)
                drt = eftp.tile([128, k_blk * 128], bf16, tag="drep")
                nc.sync.dma_start(
                    out=drt[:], in_=dstrep[:, blk * k_blk * 128:(blk + 1) * k_blk * 128]
                )

                ag0 = psG.tile([128, 128], f32, tag="agg")
                ag1 = psG.tile([128, 128], f32, tag="agg")
                pdst_blk = pdst_sb[:, blk * H:(blk + 1) * H]
                for g0 in range(0, k_blk, GCH):
                    g1 = min(g0 + GCH, k_blk)
                    gw = g1 - g0
                    c0 = blk * k_blk + g0
                    # gather Psrc[src] for these chunks
                    gt = gpool.tile([128, GCH, H], bf16, tag="gath")
                    nc.gpsimd.dma_gather(
                        gt[:, 0:gw, :],
                        pall[:],
                        gidx_s[:, c0 * 8:c0 * 8 + gw * 8],
                        num_idxs=gw * 128,
                        num_idxs_reg=gw * 128,
                        elem_size=H,
                        queue_num=qn,
                    )
                    qn = (qn + 1) % 2
                    # selection matrices for the group
                    sel4 = selp.tile([128, GCH, 128], bf16, tag="sel")
                    nc.vector.tensor_tensor(
                        out=sel4[:, 0:gw, :],
                        in0=dstrel_s[:, c0:c0 + gw].to_broadcast([128, gw, 128]),
                        in1=iotat_s[:].rearrange("p (a n) -> p a n", a=1)
                                      .to_broadcast([128, gw, 128]),
                        op=mybir.AluOpType.is_equal,
                    )
                    selT4 = selp.tile([128, GCH * 128], bf16, tag="selT")
                    nc.vector.tensor_tensor(
                        out=selT4[:, 0:gw * 128],
                        in0=iotac_s[:].to_broadcast([128, gw * 128]),
                        in1=drt[:, g0 * 128:g1 * 128],
                        op=mybir.AluOpType.is_equal,
                    )
                    # eproj + Pdst-expand accumulated in PSUM
                    pe4 = psA.tile([128, GCH * H], f32, tag="psA")
                    for k in range(gw):
                        ck = g0 + k
                        nc.tensor.matmul(
                            out=pe4[:, k * H:(k + 1) * H],
                            lhsT=et[:, ck * 128:(ck + 1) * 128],
                            rhs=w1e_s[:], start=True, stop=False,
                        )
                        nc.tensor.matmul(
                            out=pe4[:, k * H:(k + 1) * H],
                            lhsT=selT4[:, k * 128:(k + 1) * 128],
                            rhs=pdst_blk, start=False, stop=True,
                        )
                    # s = Psrc[src] + (eproj + Pdst[dst]) ; h = gelu(s)
                    s4 = hp.tile([128, GCH * H], bf16, tag="s")
                    nc.vector.tensor_add(
                        out=s4[:, 0:gw * H],
                        in0=gt[:, 0:gw, :].rearrange("p a n -> p (a n)"),
                        in1=pe4[:, 0:gw * H],
                    )
                    h4 = hp.tile([128, GCH * H], bf16, tag="h")
                    nc.scalar.activation(
                        out=h4[:, 0:gw * H], in_=s4[:, 0:gw * H],
                        func=mybir.ActivationFunctionType.Gelu_apprx_tanh,
                    )
                    # scatter: aggT[:, blk] += h.T @ sel
                    for k in range(gw):
                        ck = g0 + k
                        nc.tensor.matmul(
                            out=ag0[:], lhsT=h4[:, k * H:k * H + 128],
                            rhs=sel4[:, k, :],
                            start=(ck == 0), stop=(ck == k_blk - 1),
                        )
                        nc.tensor.matmul(
                            out=ag1[:], lhsT=h4[:, k * H + 128:(k + 1) * H],
                            rhs=sel4[:, k, :],
                            start=(ck == 0), stop=(ck == k_blk - 1),
                        )
                csl = slice(blk * 128, (blk + 1) * 128)
                nc.vector.tensor_copy(out=aggT0[:, csl], in_=ag0[:])
                nc.vector.tensor_copy(out=aggT1[:, csl], in_=ag1[:])

            # ---- stage 3: per-node MLPs (feature-major, 512-node groups) ----
            for g5 in range(NSLICE // 512):
                sl = slice(g5 * 512, (g5 + 1) * 512)
                for o in range(2):
                    osl = slice(o * 128, (o + 1) * 128)
                    # aggfinal = aggT.T@W2m + deg*b2m   (feature-major out)
                    pa = psA.tile([128, 512], f32, tag="psA")
                    nc.tensor.matmul(out=pa[:], lhsT=w2m_s[:, 0 * H + o * 128:0 * H + (o + 1) * 128],
                                     rhs=aggT0[:, sl], start=True, stop=False)
                    nc.tensor.matmul(out=pa[:], lhsT=w2m_s[:, 1 * H + o * 128:1 * H + (o + 1) * 128],
                                     rhs=aggT1[:, sl], start=False, stop=False)
                    nc.tensor.matmul(out=pa[:], lhsT=b2mr_s[:, osl],
                                     rhs=degrow_s[:, sl], start=False, stop=True)
                    dst_t = agfT0 if o == 0 else agfT1
                    nc.vector.tensor_copy(out=dst_t[:, sl], in_=pa[:])
                for o in range(2):
                    osl = slice(o * 128, (o + 1) * 128)
                    # u = gelu(concat(x, aggfinal) @ W1u + b1u)
                    pu = psA.tile([128, 512], f32, tag="psA")
                    nc.tensor.matmul(out=pu[:], lhsT=w1u_s[:, 0 * H + o * 128:0 * H + (o + 1) * 128],
                                     rhs=nfs_s[:, sl], start=True, stop=False)
                    nc.tensor.matmul(out=pu[:], lhsT=w1u_s[:, 1 * H + o * 128:1 * H + (o + 1) * 128],
                                     rhs=agfT0[:, sl], start=False, stop=False)
                    nc.tensor.matmul(out=pu[:], lhsT=w1u_s[:, 2 * H + o * 128:2 * H + (o + 1) * 128],
                                     rhs=agfT1[:, sl], start=False, stop=True)
                    dst_t = u0 if o == 0 else u1
                    nc.scalar.activation(
                        out=dst_t[:, sl], in_=pu[:],
                        func=mybir.ActivationFunctionType.Gelu_apprx_tanh,
                        bias=b1uc_s[:, o:o + 1],
                    )
            # out = u @ W2u + b2u   (token-major out per node block)
            for blk in range(NBLK):
                csl = slice(blk * 128, (blk + 1) * 128)
                po = psA.tile([128, 128], f32, tag="psA")
                nc.tensor.matmul(out=po[:], lhsT=u0[:, csl], rhs=w2u_s[:, 0:D],
                                 start=True, stop=False)
                nc.tensor.matmul(out=po[:], lhsT=u1[:, csl], rhs=w2u_s[:, D:2 * D],
                                 start=False, stop=False)
                nc.tensor.matmul(out=po[:], lhsT=onesr_s[:], rhs=b2ur_s[:],
                                 start=False, stop=True)
                oc = cp.tile([128, 128], f32, tag="ocp")
                nc.vector.tensor_copy(out=oc[:], in_=po[:])
                nc.sync.dma_start(out=out[csl, :], in_=oc[:])

    nc.finalize()
    return nc


def _prep_core_inputs(g, r, node_features, edge_indices, edge_features, k_blk, shared):
    """Host-side shard prep for core (graph g, dst-range r)."""
    nchunk = NBLK * k_blk
    ecap = nchunk * 128
    dst = edge_indices[g, :, 1]
    src = edge_indices[g, :, 0]
    lo, hi = r * NSLICE, (r + 1) * NSLICE

    mask = (dst >= lo) & (dst < hi)
    eid = np.nonzero(mask)[0]
    dloc = dst[eid] - lo
    blk_of = dloc // BLK
    order = np.argsort(blk_of, kind="stable")
    eid = eid[order]
    dloc = dloc[order]
    blk_of = blk_of[order]
    counts = np.bincount(blk_of, minlength=NBLK)

    # slot layout: block b occupies [b*k_blk*128, b*k_blk*128 + counts[b])
    slot = np.zeros(ecap, dtype=np.int64) - 1
    srcpad = np.zeros(ecap, dtype=np.int64)            # gather idx 0 for pads
    dstpad = np.zeros(ecap, dtype=np.int64)            # gather Pdst row 0 for pads
    drel = np.full(ecap, -1.0, dtype=np.float64)       # -1 => sel column all-zero
    epos = 0
    for b in range(NBLK):
        cnt = counts[b]
        s0 = b * k_blk * 128
        ids = eid[epos:epos + cnt]
        srcpad[s0:s0 + cnt] = src[ids]
        dstpad[s0:s0 + cnt] = dloc[epos:epos + cnt]
        drel[s0:s0 + cnt] = (dloc[epos:epos + cnt] - b * BLK).astype(np.float64)
        slot[s0:s0 + cnt] = ids
        epos += cnt

    # edge features (feature-major + ones row), zeros for pads
    eftc = np.zeros((F + 1, ecap), dtype=BF16)
    valid = slot >= 0
    eftc[:F, valid] = edge_features[g, slot[valid], :].T.astype(BF16)
    eftc[F, :] = BF16(1.0)

    # gather indices (src only); wrapped in 16 partitions, replicated per Q7 core
    gidxc = np.tile(srcpad.astype(np.int16).reshape(-1, 16).T, (8, 1))

    # dst_rel per chunk column, and row-replicated form for sel_T
    drelc = np.ascontiguousarray(drel.reshape(nchunk, 128).T).astype(BF16)
    drepc = np.broadcast_to(drel.astype(BF16)[None, :], (128, ecap))

    deg = np.bincount(dloc, minlength=NSLICE).astype(np.float64)
    degc = deg[None, :].astype(BF16)

    inp = dict(shared)
    inp["nft"] = shared["_nftg"][g]
    inp["nfs"] = np.ascontiguousarray(shared["_nftg"][g][:, lo:hi])
    inp["eft"] = eftc
    inp["gidx"] = gidxc
    inp["dstrel"] = drelc
    inp["dstrep"] = np.ascontiguousarray(drepc)
    inp["degrow"] = degc
    return {k: v for k, v in inp.items() if not k.startswith("_")}


def kernel(node_features, edge_indices, edge_features,
           W1m, b1m, W2m, b2m, W1u, b1u, W2u, b2u):
    node_features = np.asarray(node_features)
    edge_indices = np.asarray(edge_indices)
    edge_features = np.asarray(edge_features)

    # chunks per node block: driven by the actual max block occupancy
    dst = edge_indices[..., 1]
    blk_id = (np.arange(B)[:, None] * (NPAD // BLK)) + dst // BLK
    counts = np.bincount(blk_id.reshape(-1), minlength=B * NPAD // BLK)
    k_blk = int(np.ceil(counts.max() / 128.0))

    if k_blk not in _BUILD_CACHE:
        _BUILD_CACHE[k_blk] = _build(k_blk)
    nc = _BUILD_CACHE[k_blk]

    # node features transposed + padded, bf16, per graph
    nftg = np.zeros((B, D, NPAD), dtype=BF16)
    for g in range(B):
        nftg[g, :, :N] = np.asarray(node_features[g]).T.astype(BF16)

    iota = np.broadcast_to(np.arange(128, dtype=np.float32), (128, 128))
    shared = {
        "_nftg": nftg,
        "w1e": np.concatenate([np.asarray(W1m)[2 * D:], np.asarray(b1m)[None, :]],
                              axis=0).astype(BF16),
        "wsrc": np.asarray(W1m)[:D].astype(BF16),
        "wdst": np.asarray(W1m)[D:2 * D].astype(BF16),
        "w2m": np.asarray(W2m).reshape(2, 128, H).transpose(1, 0, 2).reshape(128, 2 * H).astype(BF16),
        "b2mr": np.asarray(b2m)[None, :].astype(BF16),
        "w1u": np.asarray(W1u).reshape(3, 128, H).transpose(1, 0, 2).reshape(128, 3 * H).astype(BF16),
        "b1uc": np.asarray(b1u).reshape(2, 128).T.astype(np.float32).copy(),
        "w2u": np.asarray(W2u).reshape(2, 128, D).transpose(1, 0, 2).reshape(128, 2 * D).astype(BF16),
        "b2ur": np.asarray(b2u)[None, :].astype(BF16),
        "onesr": np.ones((1, 128), dtype=BF16),
        "iotat": iota.astype(BF16),
        "iotac": np.arange(128, dtype=np.float32)[:, None].astype(BF16),
    }

    in_maps = []
    for c in range(NCORES):
        g, r = c // CPG, c % CPG
        in_maps.append(_prep_core_inputs(
            g, r, node_features, edge_indices, edge_features, k_blk, shared))

    global _LAST_IN_MAPS
    _LAST_IN_MAPS = in_maps
    res = run_bass_kernel_spmd(nc, in_maps, core_ids=list(range(NCORES)))

    outp = np.zeros((B, NPAD, D), dtype=np.float32)
    for c in range(NCORES):
        g, r = c // CPG, c % CPG
        outp[g, r * NSLICE:(r + 1) * NSLICE, :] = res.results[c]["out"]
    return outp[:, :N, :]
